# revision 1
# baseline (speedup 1.0000x reference)
"""DeepGT (graph transformer conv, heads=1) on 8 Trainium2 NeuronCores.

Strategy (SPMD, one Bass program on 8 cores, per-core data differs):
  - Nodes are round-robin assigned (degree-sorted) to cores; each core owns
    NPC node slots (incl. dummy padding). Edges are grouped by dst node.
  - Per layer: every core computes K|V for its nodes (bf16), AllGathers the
    interleaved KV table [C*NPC, 256]; attention runs per 128-node block:
    one indirect-DMA gathers all [128, D_b, 256] KV rows of the block's
    in-edges (512B descriptors = full DMA efficiency), logits/softmax/
    aggregation run on DVE/ACT with a pairwise in-place tree reduction,
    the residual h stays fp32 in SBUF (updated in place, transposed layout
    [feat, node] so every matmul lhsT is a plain slice).
  - Degree-sorted blocks make the per-block max degree (= gather width D_b)
    small and uniform across cores, so the shared SPMD program wastes little.
"""

import numpy as np
import ml_dtypes
from contextlib import ExitStack

import concourse.bass as bass
import concourse.tile as tile
from concourse import bacc
from concourse import mybir
from concourse.bass import IndirectOffsetOnAxis
from concourse.bass_utils import run_bass_kernel_spmd
from concourse.masks import make_identity

P = 128
C = 8  # cores

F32 = mybir.dt.float32
BF16 = mybir.dt.bfloat16
I32 = mybir.dt.int32


# ----------------------------------------------------------------------------
# Host-side graph planning
# ----------------------------------------------------------------------------

class Plan:
    pass


def make_plan(edge_index: np.ndarray, n_nodes: int) -> Plan:
    """Partition nodes across C cores, build per-core padded CSR grids.

    Round-robin over degree-descending nodes gives each core an almost
    identical degree profile, so one shared per-block gather width works
    for all cores (SPMD requires an identical instruction stream).
    """
    pl = Plan()
    src = np.asarray(edge_index[0], dtype=np.int64)
    dst = np.asarray(edge_index[1], dtype=np.int64)

    deg = np.bincount(dst, minlength=n_nodes).astype(np.int64)
    order = np.argsort(-deg, kind="stable")  # degree desc

    npc_nodes = -(-n_nodes // C)  # real nodes per core (max)
    NPC = -(-npc_nodes // P) * P  # padded slots per core
    NB = NPC // P

    # global rank r -> core r%C, slot r//C
    core_of = np.empty(n_nodes, dtype=np.int64)
    slot_of = np.empty(n_nodes, dtype=np.int64)
    r = np.arange(n_nodes)
    core_of[order] = r % C
    slot_of[order] = r // C
    new_id = core_of * NPC + slot_of  # padded global id

    # per-block shared gather width: block b holds ranks [b*P, (b+1)*P) of each
    # core = global ranks [b*P*C, (b+1)*P*C); max degree there = first one.
    deg_sorted = deg[order]
    Ddim = []
    for b in range(NB):
        g0 = b * P * C
        Ddim.append(max(1, int(deg_sorted[g0]) if g0 < n_nodes else 1))
    pl.Ddim = Ddim
    pl.SUM_D = int(np.sum(Ddim))
    pl.DMAX = int(np.max(Ddim))
    pl.NPC, pl.NB = NPC, NB

    # CSR: edges grouped by dst
    e_order = np.argsort(dst, kind="stable")
    src_by_dst = src[e_order]
    starts = np.zeros(n_nodes + 1, dtype=np.int64)
    np.cumsum(np.bincount(dst, minlength=n_nodes), out=starts[1:])

    # per-core grids
    pl.gidx = np.zeros((C, P, pl.SUM_D), dtype=np.int32)
    pl.gdeg = np.zeros((C, P, NB), dtype=np.float32)
    # node sitting at (core, block, partition) — -1 for dummy
    slot_node = np.full((C, NPC), -1, dtype=np.int64)
    slot_node[core_of, slot_of] = np.arange(n_nodes)

    for c in range(C):
        off = 0
        for b in range(NB):
            D = Ddim[b]
            for p in range(P):
                j = b * P + p
                o = slot_node[c, j]
                self_id = c * NPC + j
                if o >= 0:
                    d0 = int(starts[o])
                    dn = int(deg[o])
                    ids = new_id[src_by_dst[d0 : d0 + dn]]
                    pl.gidx[c, p, off : off + dn] = ids
                    if dn < D:
                        pl.gidx[c, p, off + dn : off + D] = self_id
                    pl.gdeg[c, p, b] = dn
                else:
                    pl.gidx[c, p, off : off + D] = self_id
            off += D

    pl.new_id = new_id
    pl.core_of = core_of
    pl.slot_of = slot_of
    return pl


# ----------------------------------------------------------------------------
# Bass program
# ----------------------------------------------------------------------------

def build_nc(pl: Plan, L: int, ODIM: int, n_cores: int = C,
             zero_bias: bool = False) -> bass.Bass:
    NPC, NB, Ddim, SUM_D, DMAX = pl.NPC, pl.NB, pl.Ddim, pl.SUM_D, pl.DMAX

    nc = bacc.Bacc("TRN2", target_bir_lowering=False, debug=False,
                   num_devices=n_cores)

    # --- DRAM parameters (per-core data fed via in_maps) ---
    xT_d = nc.dram_tensor("xT", [P, NPC], F32, kind="ExternalInput").ap()
    gidx_d = nc.dram_tensor("gidx", [P, SUM_D], I32, kind="ExternalInput").ap()
    gdeg_d = nc.dram_tensor("gdeg", [P, NB], F32, kind="ExternalInput").ap()
    mask_d = nc.dram_tensor("mask", [P, SUM_D], F32, kind="ExternalInput").ap()
    linW_d = nc.dram_tensor("linW", [P, P], F32, kind="ExternalInput").ap()
    linb_d = nc.dram_tensor("linb", [P, 1], F32, kind="ExternalInput").ap()
    wqkv_d = nc.dram_tensor("wqkv", [L, P, 3 * P], BF16, kind="ExternalInput").ap()
    ws_d = nc.dram_tensor("ws", [L, P, P], F32, kind="ExternalInput").ap()
    bqr_d = nc.dram_tensor("bqr", [L, P, P], F32, kind="ExternalInput").ap()
    bkvr_d = nc.dram_tensor("bkvr", [L, P, 2 * P], F32, kind="ExternalInput").ap()
    bs_d = nc.dram_tensor("bs", [P, L], F32, kind="ExternalInput").ap()
    fcW_d = nc.dram_tensor("fcW", [P, ODIM], F32, kind="ExternalInput").ap()
    fcbr_d = nc.dram_tensor("fcbr", [P, ODIM], F32, kind="ExternalInput").ap()
    out_d = nc.dram_tensor("out", [NPC, ODIM], F32, kind="ExternalOutput").ap()

    HNPC = NPC // 2
    kv_local = nc.dram_tensor("kv_local", [NPC, 2 * P], BF16).ap()
    # Shared-output AllGather is the fast collective path, but cores are
    # PAIR-shared on this platform: the pair partner's next-layer AllGather
    # would overwrite the table this core still gathers from. Alternate
    # between two tables by layer parity to break the race.
    kv_fullA = nc.dram_tensor("kv_fullA", [n_cores * NPC, 2 * P], BF16,
                              addr_space="Shared").ap()
    kv_fullB = nc.dram_tensor("kv_fullB", [n_cores * NPC, 2 * P], BF16,
                              addr_space="Shared").ap()

    with tile.TileContext(nc) as tc, ExitStack() as ctx:
        cp = ctx.enter_context(tc.tile_pool(name="const", bufs=1))
        hp = ctx.enter_context(tc.tile_pool(name="h", bufs=1))
        xp = ctx.enter_context(tc.tile_pool(name="x", bufs=3))
        kvp = ctx.enter_context(tc.tile_pool(name="kv", bufs=2))
        prp = ctx.enter_context(tc.tile_pool(name="prod", bufs=3))
        sm = ctx.enter_context(tc.tile_pool(name="small", bufs=3))

        # --- constants ---
        idx_sb = cp.tile([P, SUM_D], I32)
        nc.sync.dma_start(out=idx_sb[:], in_=gidx_d[:])
        mask_sb = cp.tile([P, SUM_D], F32)
        nc.sync.dma_start(out=mask_sb[:], in_=mask_d[:])
        linW_raw = cp.tile([P, P], F32)
        nc.sync.dma_start(out=linW_raw[:], in_=linW_d[:])
        # funnel through ACT so the first matmul's lhsT dep is an engine sem,
        # not a second DMA-queue wait (walrus LW struct holds only one)
        linW_sb = cp.tile([P, P], F32)
        nc.scalar.activation(linW_sb[:], linW_raw[:],
                             mybir.ActivationFunctionType.Copy)
        linb_sb = cp.tile([P, 1], F32)
        nc.sync.dma_start(out=linb_sb[:], in_=linb_d[:])
        fcW_sb = cp.tile([P, ODIM], F32)
        nc.sync.dma_start(out=fcW_sb[:], in_=fcW_d[:])
        fcbr_sb = cp.tile([P, ODIM], F32)
        nc.sync.dma_start(out=fcbr_sb[:], in_=fcbr_d[:])
        bs_sb = cp.tile([P, L], F32)
        nc.sync.dma_start(out=bs_sb[:], in_=bs_d[:])
        wqkv_sb, ws_sb, bqr_sb, bkvr_sb = [], [], [], []
        for l in range(L):
            t = cp.tile([P, 3 * P], BF16, tag=f"wqkv{l}")
            nc.sync.dma_start(out=t[:], in_=wqkv_d[l])
            wqkv_sb.append(t)
            t = cp.tile([P, P], F32, tag=f"ws{l}")
            nc.sync.dma_start(out=t[:], in_=ws_d[l])
            ws_sb.append(t)
            t = cp.tile([P, P], F32, tag=f"bqr{l}")
            nc.sync.dma_start(out=t[:], in_=bqr_d[l])
            bqr_sb.append(t)
            t = cp.tile([P, 2 * P], F32, tag=f"bkvr{l}")
            nc.sync.dma_start(out=t[:], in_=bkvr_d[l])
            bkvr_sb.append(t)
        ident = cp.tile([P, P], F32)
        make_identity(nc, ident[:])

        hT = hp.tile([P, NPC], F32)     # residual stream, [feat, node]
        hTb = hp.tile([P, NPC], BF16)   # bf16 shadow for matmul inputs



        def bcast_f(ap2d, D):
            # [P, F] -> [P, D, F] (middle broadcast)
            return ap2d.rearrange("p (o f) -> p o f", o=1).to_broadcast(
                [P, D, ap2d.shape[1]])

        def bcast_l(ap2d, F):
            # [P, D] -> [P, D, F] (last broadcast)
            return ap2d.rearrange("p (d o) -> p d o", o=1).to_broadcast(
                [P, ap2d.shape[1], F])

        # --- input projection: hT = linW.T @ xT (+ linb), bf16 shadow ---
        CH = 512
        with tc.tile_pool(name="ppb", bufs=2, space="PSUM") as pp_big:
            for off in range(0, NPC, CH):
                csz = min(CH, NPC - off)
                xt = xp.tile([P, CH], F32, tag="xt")
                nc.sync.dma_start(out=xt[:, :csz], in_=xT_d[:, off:off + csz])
                pb = pp_big.tile([P, CH], F32, tag="pbig")
                nc.tensor.matmul(pb[:, :csz], lhsT=linW_sb[:], rhs=xt[:, :csz],
                                 start=True, stop=True)
                nc.scalar.activation(hT[:, off:off + csz], pb[:, :csz],
                                     mybir.ActivationFunctionType.Identity,
                                     bias=linb_sb[:, 0:1])
                nc.vector.tensor_copy(hTb[:, off:off + csz], hT[:, off:off + csz])

        # --- layers ---
        lp = ctx.enter_context(tc.tile_pool(name="ppkv", bufs=2, space="PSUM"))
        pp_kv = lp
        pp_sm = ctx.enter_context(tc.tile_pool(name="ppsm", bufs=2, space="PSUM"))
        for l in range(L):
            # K|V for own nodes -> kv_local, then AllGather -> kv_full
            for b in range(NB):
                blk = slice(b * P, (b + 1) * P)
                pkv = pp_kv.tile([P, 2 * P], F32, tag="pkv")
                nc.tensor.matmul(pkv[:], lhsT=hTb[:, blk],
                                 rhs=wqkv_sb[l][:, P:3 * P],
                                 start=True, stop=True)
                kvs = sm.tile([P, 2 * P], BF16, tag="kvs")
                if zero_bias:
                    nc.scalar.activation(kvs[:], pkv[:],
                                         mybir.ActivationFunctionType.Copy)
                else:
                    nc.vector.tensor_tensor(out=kvs[:], in0=pkv[:],
                                            in1=bkvr_sb[l][:],
                                            op=mybir.AluOpType.add)
                nc.sync.dma_start(out=kv_local[blk, :], in_=kvs[:])

            kv_flat = kv_fullA if l % 2 == 0 else kv_fullB
            nc.gpsimd.collective_compute(
                "AllGather", mybir.AluOpType.bypass,
                replica_groups=[list(range(n_cores))],
                ins=[kv_local[:]], outs=[kv_flat[:]])

            # attention per block
            off = 0
            for b in range(NB):
                blk = slice(b * P, (b + 1) * P)
                D = Ddim[b]
                kvb = kvp.tile([P, DMAX, 2 * P], BF16, tag="kvb")
                for d in range(D):
                    # HW indirect DMA honors exactly one index per partition
                    nc.gpsimd.indirect_dma_start(
                        out=kvb[:, d, :], out_offset=None, in_=kv_flat[:, :],
                        in_offset=IndirectOffsetOnAxis(
                            ap=idx_sb[:, off + d:off + d + 1], axis=0))

                pq = pp_sm.tile([P, P], F32, tag="pq")
                nc.tensor.matmul(pq[:], lhsT=hTb[:, blk],
                                 rhs=wqkv_sb[l][:, 0:P], start=True, stop=True)
                qb = sm.tile([P, P], BF16, tag="qb")
                if zero_bias:
                    nc.scalar.activation(qb[:], pq[:],
                                         mybir.ActivationFunctionType.Copy)
                else:
                    nc.vector.tensor_tensor(out=qb[:], in0=pq[:],
                                            in1=bqr_sb[l][:],
                                            op=mybir.AluOpType.add)

                prod = prp.tile([P, DMAX, P], BF16, tag="prod")
                nc.vector.tensor_tensor(out=prod[:, :D, :],
                                        in0=kvb[:, :D, 0:P],
                                        in1=bcast_f(qb[:], D),
                                        op=mybir.AluOpType.mult)
                Lb = sm.tile([P, DMAX], F32, tag="Lb")
                nc.vector.reduce_sum(Lb[:, :D], prod[:, :D, :],
                                     axis=mybir.AxisListType.X)

                mneg = sm.tile([P, 1], F32, tag="mneg")
                nc.vector.reduce_max(mneg[:], Lb[:, :D],
                                     axis=mybir.AxisListType.X, negate=True)
                et = sm.tile([P, DMAX], F32, tag="et")
                nc.scalar.activation(et[:, :D], Lb[:, :D],
                                     mybir.ActivationFunctionType.Exp,
                                     bias=mneg[:, 0:1])
                em = sm.tile([P, DMAX], F32, tag="em")
                den = sm.tile([P, 1], F32, tag="den")
                nc.vector.scalar_tensor_tensor(
                    out=em[:, :D], in0=et[:, :D], scalar=1.0,
                    in1=mask_sb[:, off:off + D], op0=mybir.AluOpType.mult,
                    op1=mybir.AluOpType.mult, accum_out=den[:])
                den2 = sm.tile([P, 1], F32, tag="den2")
                nc.vector.tensor_scalar(out=den2[:], in0=den[:],
                                        scalar1=1e-30, scalar2=None,
                                        op0=mybir.AluOpType.add)
                rden = sm.tile([P, 1], F32, tag="rden")
                nc.vector.reciprocal(rden[:], den2[:])
                en = sm.tile([P, DMAX], BF16, tag="en")
                nc.vector.tensor_scalar(out=en[:, :D], in0=em[:, :D],
                                        scalar1=rden[:, 0:1], scalar2=None,
                                        op0=mybir.AluOpType.mult)

                nc.vector.tensor_tensor(out=prod[:, :D, :],
                                        in0=kvb[:, :D, P:2 * P],
                                        in1=bcast_l(en[:, :D], P),
                                        op=mybir.AluOpType.mult)
                dd = D
                while dd > 1:
                    h2 = (dd + 1) // 2
                    r = dd - h2
                    nc.vector.tensor_tensor(out=prod[:, :r, :],
                                            in0=prod[:, :r, :],
                                            in1=prod[:, h2:dd, :],
                                            op=mybir.AluOpType.add)
                    dd = h2

                ps = pp_sm.tile([P, P], F32, tag="ps")
                nc.tensor.matmul(ps[:], lhsT=hT[:, blk], rhs=ws_sb[l][:],
                                 start=True, stop=True)
                ts = sm.tile([P, P], F32, tag="ts")
                nc.vector.tensor_tensor(
                    out=ts[:], in0=prod[:, 0:1, :].rearrange("p o f -> p (o f)"),
                    in1=ps[:], op=mybir.AluOpType.add)
                pt = pp_sm.tile([P, P], F32, tag="pt")
                nc.tensor.transpose(out=pt[:], in_=ts[:], identity=ident[:])

                if l < L - 1:
                    # ELU(y) = relu(y) + min(exp(y)-1, 0), y = pt + bs
                    es = sm.tile([P, P], F32, tag="es")
                    nc.scalar.activation(es[:], pt[:],
                                         mybir.ActivationFunctionType.Exp,
                                         bias=bs_sb[:, l:l + 1])
                    rs = sm.tile([P, P], F32, tag="rs")
                    nc.scalar.activation(rs[:], pt[:],
                                         mybir.ActivationFunctionType.Relu,
                                         bias=bs_sb[:, l:l + 1])
                    mp = sm.tile([P, P], F32, tag="mp")
                    nc.vector.tensor_scalar(out=mp[:], in0=es[:],
                                            scalar1=1.0, scalar2=0.0,
                                            op0=mybir.AluOpType.subtract,
                                            op1=mybir.AluOpType.min)
                    nc.vector.tensor_tensor(out=hT[:, blk], in0=rs[:],
                                            in1=mp[:],
                                            op=mybir.AluOpType.add)
                    nc.vector.tensor_copy(hTb[:, blk], hT[:, blk])
                else:
                    nc.scalar.activation(hT[:, blk], pt[:],
                                         mybir.ActivationFunctionType.Identity,
                                         bias=bs_sb[:, l:l + 1])
                off += D

        # --- classifier + log_softmax ---
        for b in range(NB):
            blk = slice(b * P, (b + 1) * P)
            po = pp_sm.tile([P, ODIM], F32, tag="pq")
            nc.tensor.matmul(po[:], lhsT=hT[:, blk], rhs=fcW_sb[:],
                             start=True, stop=True)
            if zero_bias:
                Lo = po
            else:
                Lo = sm.tile([P, ODIM], F32, tag="Lo")
                nc.vector.tensor_tensor(out=Lo[:], in0=po[:], in1=fcbr_sb[:],
                                        op=mybir.AluOpType.add)
            mn = sm.tile([P, 1], F32, tag="mn")
            nc.vector.reduce_max(mn[:], Lo[:], axis=mybir.AxisListType.X,
                                 negate=True)
            eo = sm.tile([P, ODIM], F32, tag="eo")
            dn = sm.tile([P, 1], F32, tag="dn")
            nc.scalar.activation(eo[:], Lo[:],
                                 mybir.ActivationFunctionType.Exp,
                                 bias=mn[:, 0:1], accum_out=dn[:])
            lnd = sm.tile([P, 1], F32, tag="lnd")
            nc.scalar.activation(lnd[:], dn[:],
                                 mybir.ActivationFunctionType.Ln)
            cc = sm.tile([P, 1], F32, tag="cc")
            nc.vector.tensor_tensor(out=cc[:], in0=mn[:], in1=lnd[:],
                                    op=mybir.AluOpType.subtract)
            oo = sm.tile([P, ODIM], F32, tag="oo")
            nc.vector.tensor_scalar(out=oo[:], in0=Lo[:],
                                    scalar1=cc[:, 0:1], scalar2=None,
                                    op0=mybir.AluOpType.add)
            nc.sync.dma_start(out=out_d[blk, :], in_=oo[:])

    nc.compile()
    return nc


# ----------------------------------------------------------------------------
# Host-side input packing
# ----------------------------------------------------------------------------

def make_in_maps(pl: Plan, x, lin_W, lin_b, Wq, bq, Wk, bk, Wv, bv, Ws, bs,
                 fc_W, fc_b, n_cores: int = C):
    L = Wq.shape[0]
    HD = Wq.shape[1]
    ODIM = fc_W.shape[1]
    NPC, NB, DMAX, SUM_D = pl.NPC, pl.NB, pl.DMAX, pl.SUM_D
    scale = np.float32(1.0 / np.sqrt(HD))

    wqkv = np.concatenate([Wq * scale, Wk, Wv], axis=2).astype(ml_dtypes.bfloat16)
    bqr = np.broadcast_to((bq * scale)[:, None, :], (L, P, HD)).astype(np.float32)
    bkv = np.concatenate([bk, bv], axis=1)  # [L, 2H]
    bkvr = np.broadcast_to(bkv[:, None, :], (L, P, 2 * HD)).astype(np.float32)
    bs_cols = np.ascontiguousarray(bs.T.astype(np.float32))  # [H, L] -> [P, L]
    fcbr = np.broadcast_to(fc_b[None, :], (P, ODIM)).astype(np.float32)
    lin_bc = np.ascontiguousarray(lin_b.astype(np.float32)[:, None])  # [P,1]

    shared = {
        "linW": np.ascontiguousarray(lin_W.astype(np.float32)),
        "linb": lin_bc,
        "wqkv": np.ascontiguousarray(wqkv),
        "ws": np.ascontiguousarray(Ws.astype(np.float32)),
        "bqr": np.ascontiguousarray(bqr),
        "bkvr": np.ascontiguousarray(bkvr),
        "bs": bs_cols,
        "fcW": np.ascontiguousarray(fc_W.astype(np.float32)),
        "fcbr": np.ascontiguousarray(fcbr),
    }

    n_nodes = x.shape[0]
    in_maps = []
    for c in range(C if n_cores == C else n_cores):
        # xT: [P, NPC] — x rows of this core's slots, transposed
        xT = np.zeros((P, NPC), dtype=np.float32)
        sel = pl.core_of == c
        xT[:, pl.slot_of[sel]] = x[sel].T.astype(np.float32)
        m = dict(shared)
        m["xT"] = xT
        m["gidx"] = np.ascontiguousarray(pl.gidx[c])
        m["gdeg"] = np.ascontiguousarray(pl.gdeg[c])
        msk = np.zeros((P, pl.SUM_D), dtype=np.float32)
        off = 0
        for b in range(NB):
            D = pl.Ddim[b]
            msk[:, off:off + D] = (np.arange(D)[None, :]
                                   < pl.gdeg[c][:, b:b + 1])
            off += D
        m["mask"] = msk
        in_maps.append(m)
    return in_maps


def unpack_out(pl: Plan, results, n_nodes: int, ODIM: int):
    out = np.empty((n_nodes, ODIM), dtype=np.float32)
    for c in range(C):
        o = results[c]["out"]  # [NPC, ODIM]
        sel = pl.core_of == c
        out[sel] = o[pl.slot_of[sel]]
    return out


# ----------------------------------------------------------------------------
# Entry point
# ----------------------------------------------------------------------------

def kernel(**inputs) -> np.ndarray:
    x = np.asarray(inputs["x"], dtype=np.float32)
    edge_index = np.asarray(inputs["edge_index"], dtype=np.int32)
    args = {k: np.asarray(v) for k, v in inputs.items() if k not in ("x", "edge_index")}

    n_nodes = x.shape[0]
    L = args["Wq"].shape[0]
    ODIM = args["fc_W"].shape[1]

    pl = make_plan(edge_index, n_nodes)
    zb = all(not np.any(args[k]) for k in ("lin_b", "bq", "bk", "bv", "bs", "fc_b"))
    nc = build_nc(pl, L, ODIM, zero_bias=zb)
    in_maps = make_in_maps(pl, x, args["lin_W"], args["lin_b"],
                           args["Wq"], args["bq"], args["Wk"], args["bk"],
                           args["Wv"], args["bv"], args["Ws"], args["bs"],
                           args["fc_W"], args["fc_b"])
    res = run_bass_kernel_spmd(nc, in_maps, list(range(C)))
    return unpack_out(pl, res.results, n_nodes, ODIM)

